# revision 1
# baseline (speedup 1.0000x reference)
"""Causal self-attention (B=4, T=2048, C=1024, H=16, D=64) on 8 Trainium2
NeuronCores.

Sharding: core c = (batch b = c//2, head-group g = c%2 of 8 heads).
Each core computes q/k/v projections for its 8 heads, causal flash-style
attention in S^T = [tk, tq] layout (softmax denominators via a ones-row
appended to V; exp on ScalarE; additive -1e30 mask on diagonal 128-blocks;
lower-left tile skipping), then a partial o_proj. Host sums the two
head-group partials per batch.

Precision: fp32r (TensorE's full-rate rounded-fp32 mode) for projections and
q@k; fp16 for P/V/O/o_proj (validated ~5e-4 rel err vs fp32 reference).
"""

from contextlib import ExitStack

import numpy as np

import concourse.tile as tile
from concourse import bacc, mybir
from concourse.bass_utils import run_bass_kernel_spmd

F32 = mybir.dt.float32
F32R = mybir.dt.float32r
FP16 = mybir.dt.float16
EXP = mybir.ActivationFunctionType.Exp

B, T, C, NHEAD, D = 4, 2048, 1024, 16, 64
H = 8                      # heads per core
HD = H * D                 # 512
NT = T // 128              # 16 tk tiles
NJ = T // 512              # 4 tq chunks
NC = C // 128              # 8 contraction chunks
NM = HD // 128             # 4 qT/kT partition tiles
NYN = C // 512             # 2 o_proj N chunks
DJ = 4                     # tk tiles per tq chunk


def build_nc(loop_k=0):
    nc = bacc.Bacc("TRN2", target_bir_lowering=False, debug=False,
                   enable_asserts=False, num_devices=8)

    xT = nc.dram_tensor("xT", [C, T], F32R, kind="ExternalInput").ap()
    wqT = nc.dram_tensor("wqT", [C, HD], F32R, kind="ExternalInput").ap()
    wkT = nc.dram_tensor("wkT", [C, HD], F32R, kind="ExternalInput").ap()
    wvT = nc.dram_tensor("wvT", [C, HD], F32R, kind="ExternalInput").ap()
    woT = nc.dram_tensor("woT", [HD, C], FP16, kind="ExternalInput").ap()
    maskd = nc.dram_tensor("mask", [128, 128], F32, kind="ExternalInput").ap()
    y = nc.dram_tensor("y", [T, C], F32, kind="ExternalOutput").ap()

    with tile.TileContext(nc) as tc:
        with ExitStack() as ctx:
            if loop_k:
                ctx.enter_context(tc.For_i(0, loop_k, 1))
            _body(tc, xT, wqT, wkT, wvT, woT, maskd, y)
    nc.compile()
    return nc


def _body(tc, xT, wqT, wkT, wvT, woT, maskd, y):
    nc = tc.nc
    with ExitStack() as ctx:
        ctx.enter_context(nc.allow_low_precision(reason="fp32r/fp16 pipeline"))
        pers = ctx.enter_context(tc.tile_pool(name="pers", bufs=1))
        qT = [pers.tile([128, T], F32R, tag=f"qT{m}", name=f"qT{m}")
              for m in range(NM)]
        kT = [pers.tile([128, T], F32R, tag=f"kT{m}", name=f"kT{m}")
              for m in range(NM)]
        Vs = [pers.tile([128, H * 65], FP16, tag=f"Vs{t}", name=f"Vs{t}")
              for t in range(NT)]
        OT = [pers.tile([64, T], FP16, tag=f"OT{h}", name=f"OT{h}")
              for h in range(H)]
        maskT = pers.tile([128, 128], F32, tag="mask", name="maskT")
        ones_t = pers.tile([65, 64], F32R, tag="ones", name="ones_t")

        nc.sync.dma_start(maskT[:], maskd[:])
        ones_f = pers.tile([65, 64], F32, tag="ones_f", name="ones_f")
        nc.vector.memset(ones_f[:], 1.0)
        nc.vector.tensor_copy(ones_t[:], ones_f[:])

        # ---------------- stage A: projections ----------------
        with ExitStack() as actx:
            wpool = actx.enter_context(tc.tile_pool(name="wpool", bufs=1))
            xpool = actx.enter_context(tc.tile_pool(name="xpool", bufs=2 * NC))
            psA = actx.enter_context(
                tc.tile_pool(name="psA", bufs=4, space="PSUM"))
            wq = [wpool.tile([128, HD], F32R, tag=f"wq{k}", name=f"wq{k}")
                  for k in range(NC)]
            wk = [wpool.tile([128, HD], F32R, tag=f"wk{k}", name=f"wk{k}")
                  for k in range(NC)]
            wv = [wpool.tile([128, HD], F32R, tag=f"wv{k}", name=f"wv{k}")
                  for k in range(NC)]
            for k in range(NC):
                nc.sync.dma_start(wq[k][:], wqT[128 * k:128 * k + 128, :])
                nc.sync.dma_start(wk[k][:], wkT[128 * k:128 * k + 128, :])
                nc.sync.dma_start(wv[k][:], wvT[128 * k:128 * k + 128, :])

            for tc4 in range(NJ):
                tsl = slice(512 * tc4, 512 * tc4 + 512)
                xt = []
                for k in range(NC):
                    t_ = xpool.tile([128, 512], F32R, tag="xt", name="xt")
                    nc.sync.dma_start(t_[:], xT[128 * k:128 * k + 128, tsl])
                    xt.append(t_)
                for dst, w in ((qT, wq), (kT, wk)):
                    for m in range(NM):
                        ps = psA.tile([128, 512], F32, tag="psA", name="psA")
                        for k in range(NC):
                            nc.tensor.matmul(
                                ps[:], w[k][:, 128 * m:128 * m + 128],
                                xt[k][:], start=(k == 0), stop=(k == NC - 1))
                        nc.vector.tensor_copy(dst[m][:, tsl], ps[:])
                for tt in range(4):
                    t_idx = 4 * tc4 + tt
                    ps = psA.tile([128, HD], F32, tag="psV", name="psV")
                    for k in range(NC):
                        nc.tensor.matmul(
                            ps[:], xt[k][:, 128 * tt:128 * tt + 128],
                            wv[k][:], start=(k == 0), stop=(k == NC - 1))
                    dst_ap = Vs[t_idx][:].rearrange("p (h e) -> p h e", e=65)
                    nc.vector.tensor_copy(
                        dst_ap[:, :, 0:64],
                        ps[:].rearrange("p (h e) -> p h e", e=64))
                    nc.vector.memset(dst_ap[:, :, 64:65], 1.0)

        # ---------------- stages B+C ----------------
        with ExitStack() as bctx:
            wopool = bctx.enter_context(tc.tile_pool(name="wopool", bufs=1))
            ppool = bctx.enter_context(tc.tile_pool(name="ppool", bufs=4))
            rpool = bctx.enter_context(tc.tile_pool(name="rpool", bufs=2))
            psS = bctx.enter_context(
                tc.tile_pool(name="psS", bufs=4, space="PSUM"))
            psO = bctx.enter_context(
                tc.tile_pool(name="psO", bufs=2, space="PSUM"))
            psR = bctx.enter_context(
                tc.tile_pool(name="psR", bufs=2, space="PSUM"))

            WoTh = [wopool.tile([64, C], FP16, tag=f"Wo{h}", name=f"Wo{h}")
                    for h in range(H)]
            for h in range(H):
                nc.sync.dma_start(WoTh[h][:], woT[64 * h:64 * h + 64, :])

            for h in range(H):
                pb = 64 * (h % 2)
                kTh = kT[h // 2]
                qTh = qT[h // 2]
                for j in range(NJ):
                    i_max = DJ * j + DJ - 1
                    O_ps = psO.tile([65, 512], F32, tag="O", name="Ops")
                    for i in range(i_max + 1):
                        mloc = i - DJ * j
                        off = 128 * mloc if mloc > 0 else 0
                        w = 512 - off
                        S_ps = psS.tile([128, 512], F32, tag="S", name="Sps")
                        nc.tensor.matmul(
                            S_ps[:, off:off + w],
                            kTh[pb:pb + 64, 128 * i:128 * i + 128],
                            qTh[pb:pb + 64, 512 * j + off:512 * j + off + w],
                            start=True, stop=True)
                        if mloc >= 0:
                            nc.vector.tensor_add(
                                S_ps[:, off:off + 128],
                                S_ps[:, off:off + 128], maskT[:])
                        P = ppool.tile([128, 512], FP16, tag="P", name="P")
                        nc.scalar.activation(P[:, 0:w], S_ps[:, off:off + w],
                                             EXP, scale=0.125)
                        nc.tensor.matmul(
                            O_ps[:, off:off + w],
                            Vs[i][:, 65 * h:65 * h + 65], P[:, 0:w],
                            start=(i == 0), stop=(i == i_max))
                    rt = rpool.tile([65, 512], F32R, tag="r", name="rt")
                    nc.vector.reciprocal(rt[64:65, :], O_ps[64:65, :])
                    R_ps = psR.tile([64, 512], F32, tag="R", name="Rps")
                    nc.tensor.matmul(R_ps[:], ones_t[64:65, 0:64],
                                     rt[64:65, :], start=True, stop=True)
                    Rs = rpool.tile([64, 512], F32, tag="Rs", name="Rs")
                    nc.vector.tensor_copy(Rs[:], R_ps[:])
                    nc.vector.tensor_mul(OT[h][:, 512 * j:512 * j + 512],
                                         O_ps[0:64, :], Rs[:])

            # ---------------- stage C: o_proj ----------------
            ypool = bctx.enter_context(tc.tile_pool(name="ypool", bufs=3))
            for m in range(NT):
                for n in range(NYN):
                    y_ps = psS.tile([128, 512], F32, tag="S", name="Sps")
                    for h in range(H):
                        nc.tensor.matmul(
                            y_ps[:], OT[h][:, 128 * m:128 * m + 128],
                            WoTh[h][:, 512 * n:512 * n + 512],
                            start=(h == 0), stop=(h == H - 1))
                    ysb = ypool.tile([128, 512], F32, tag="y", name="ysb")
                    nc.scalar.copy(ysb[:], y_ps[:])
                    nc.sync.dma_start(
                        y[128 * m:128 * m + 128, 512 * n:512 * n + 512],
                        ysb[:])


_NC_CACHE = {}


def _get_nc(loop_k=0):
    if loop_k not in _NC_CACHE:
        _NC_CACHE[loop_k] = build_nc(loop_k)
    return _NC_CACHE[loop_k]


_TRI_MASK = np.where(np.arange(128)[None, :] >= np.arange(128)[:, None],
                     np.float32(0), np.float32(-1e30))


def make_in_maps(x, Wq, Wk, Wv, Wo):
    x = np.asarray(x, dtype=np.float32)
    Wq = np.asarray(Wq, dtype=np.float32)
    Wk = np.asarray(Wk, dtype=np.float32)
    Wv = np.asarray(Wv, dtype=np.float32)
    Wo = np.asarray(Wo, dtype=np.float32)
    xTs = [np.ascontiguousarray(x[b].T) for b in range(B)]
    in_maps = []
    for c in range(8):
        b, g = c // 2, c % 2
        sl = slice(HD * g, HD * g + HD)
        in_maps.append({
            "xT": xTs[b],
            "wqT": np.ascontiguousarray(Wq[sl, :].T),
            "wkT": np.ascontiguousarray(Wk[sl, :].T),
            "wvT": np.ascontiguousarray(Wv[sl, :].T),
            "woT": np.ascontiguousarray(Wo[:, sl].T).astype(np.float16),
            "mask": _TRI_MASK,
        })
    return in_maps


def kernel(x, Wq, Wk, Wv, Wo):
    nc = _get_nc()
    in_maps = make_in_maps(x, Wq, Wk, Wv, Wo)
    res = run_bass_kernel_spmd(nc, in_maps, core_ids=list(range(8)))
    out = np.empty((B, T, C), dtype=np.float32)
    for b in range(B):
        out[b] = res.results[2 * b]["y"] + res.results[2 * b + 1]["y"]
    return out
